# revision 35
# baseline (speedup 1.0000x reference)
"""Trainium2 Bass kernel for nn_Critic (additive-attention critic network).

Math (per sample, weight folding done on host):
  A    = UA @ x.T + biasA          UA = W_a@W_s           [E, N]
  u1   = v_a . tanh(A)                                    [N]
  a    = softmax(u1)  (constant-shift C1, exact softmax)
  G    = U1 @ x.T + U2 @ (x.T * a) + biasG                [E, N]
  u2   = v_c . tanh(G);  p = softmax(u2)
  y    = x.T @ p   (h_i = W_s@y + b_s since sum(p)=1)
  v    = W2 @ relu(W1 @ (W_s@y + b_s) + b1) + b2

Layout strategy:
  - x transposed on host to xT [S=128, N=2048] bf16 per sample: all
    device DMAs contiguous, no DMA transpose.
  - 4 samples per group; score rows at PSUM partitions {0,32,64,96} of
    one [128, N] f32 tile -> one batched Exp per group per branch.
  - Softmax-weight broadcasts via DRAM round-trip (row -> scratch DRAM ->
    stride-0 partition-broadcast read back): PSUM stays free for matmuls,
    DVE gets 2x bf16 SBUF mode for the xT*a multiply.
  - Groups are software-pipelined: branch-1 of group g+1 is emitted
    before branch-2 of group g so every engine FIFO (and every tile-pool
    ring) always has ready work while group g waits on its softmax
    round-trip.  Post-Exp DVE work (recip/normalize) is emitted after
    the next group's xa multiplies to keep the DVE FIFO unblocked.

Sharding: data-parallel over batch, 16 samples per core on 8 cores.
"""
import sys
import numpy as np

sys.path.insert(0, "/opt/trn_rl_repo")
import ml_dtypes  # noqa: E402
import concourse.bass as bass  # noqa: E402
import bass_rust  # noqa: E402
import concourse.bacc as bacc  # noqa: E402
import concourse.mybir as mybir  # noqa: E402
import concourse.tile as tile  # noqa: E402
from concourse.bass_utils import run_bass_kernel_spmd  # noqa: E402
from contextlib import ExitStack  # noqa: E402

B, N, S, E = 128, 2048, 128, 128
NCORES, BLOC = 8, 16
NGROUPS, GSZ = 4, 4
H = N // 2  # 1024
bf16, f32 = mybir.dt.bfloat16, mybir.dt.float32
AF, ALU = mybir.ActivationFunctionType, mybir.AluOpType

_cache = {}


def _build():
    nc = bacc.Bacc("TRN2", target_bir_lowering=False, debug=False, num_devices=NCORES)
    x_d = nc.dram_tensor("x", [BLOC, S, N], bf16, kind="ExternalInput")
    uaT_d = nc.dram_tensor("uaT", [S, E], bf16, kind="ExternalInput")
    u1T_d = nc.dram_tensor("u1T", [S, E], bf16, kind="ExternalInput")
    u2T_d = nc.dram_tensor("u2T", [S, E], bf16, kind="ExternalInput")
    va_d = nc.dram_tensor("va", [E, 1], bf16, kind="ExternalInput")
    vc_d = nc.dram_tensor("vc", [E, 1], bf16, kind="ExternalInput")
    wsT_d = nc.dram_tensor("wsT", [S, E], f32, kind="ExternalInput")
    w1T_d = nc.dram_tensor("w1T", [E, E], f32, kind="ExternalInput")
    w2T_d = nc.dram_tensor("w2T", [E, 1], f32, kind="ExternalInput")
    # bias columns: 0 biasA, 1 biasG, 2 -C1, 3 -C2, 4 b_s, 5 b1, 6 b2
    bi_d = nc.dram_tensor("bi", [128, 8], f32, kind="ExternalInput")
    # broadcast scratch: row (g*2+branch)*4 + i holds sample (4g+i)'s weights
    scr_d = nc.dram_tensor("scr", [NGROUPS * 2 * GSZ, N], bf16, kind="Internal")
    v_out = nc.dram_tensor("v", [1, BLOC], f32, kind="ExternalOutput")

    with tile.TileContext(nc) as tc, ExitStack() as ctx:
        cst = ctx.enter_context(tc.tile_pool(name="cst", bufs=1))
        xp = ctx.enter_context(tc.tile_pool(name="xp", bufs=1))
        tp = ctx.enter_context(tc.tile_pool(name="tp", bufs=6))
        bp = ctx.enter_context(tc.tile_pool(name="bp", bufs=9))
        ep = ctx.enter_context(tc.tile_pool(name="ep", bufs=2))
        sp = ctx.enter_context(tc.tile_pool(name="sp", bufs=8))
        pm = ctx.enter_context(tc.tile_pool(name="pm", bufs=2, space="PSUM"))
        pu = ctx.enter_context(tc.tile_pool(name="pu", bufs=1, space="PSUM"))

        # preload the exp/tanh ACT table set during the DMA ramp: first
        # activation in program order depends only on an on-chip memset
        warm0 = sp.tile([128, 1], f32, tag="z", name="warm0")
        warm1 = sp.tile([128, 1], f32, tag="z", name="warm1")
        nc.vector.memset(warm0[:], 0.0)
        nc.scalar.activation(warm1[:], warm0[:], AF.Tanh)
        ones = cst.tile([128, 128], bf16)
        nc.gpsimd.memset(ones[:], 1.0)

        uaT = cst.tile([S, E], bf16)
        nc.sync.dma_start(uaT[:], uaT_d.ap())
        bi = cst.tile([128, 8], f32)
        nc.sync.dma_start(bi[:], bi_d.ap())
        ys = cst.tile([128, BLOC], f32)

        xts = [None] * BLOC

        def load_x(g, split_first=False):
            for s in range(g * GSZ, (g + 1) * GSZ):
                xt = xp.tile([S, N], bf16, tag=f"x{s}", bufs=1, name=f"x{s}")
                if split_first and s == g * GSZ:
                    nc.sync.dma_start(xt[:, 0:H], x_d.ap()[s][:, 0:H])
                    nc.sync.dma_start(xt[:, H:N], x_d.ap()[s][:, H:N])
                else:
                    nc.sync.dma_start(xt[:], x_d.ap()[s])
                xts[s] = xt

        # per-group state carried between pipeline stages
        st = [dict() for _ in range(NGROUPS)]

        def p1_main(g, inject=None):
            """Branch 1 matmuls/tanh/scores for group g."""
            smp = list(range(g * GSZ, (g + 1) * GSZ))
            u1g = pu.tile([128, N], f32, tag="u", name=f"u1g{g}")
            tas = []
            for i, s in enumerate(smp):
                ta = tp.tile([128, N], bf16, tag="tanh", name=f"ta{s}")
                for h in range(2):
                    mm = pm.tile([128, H], f32, tag="mm", name=f"Amm{s}_{h}")
                    for q in range(2):
                        sl = slice(H * h + 512 * q, H * h + 512 * (q + 1))
                        nc.tensor.matmul(mm[:, 512 * q:512 * (q + 1)], uaT[:],
                                         xts[s][:, sl], start=True, stop=True)
                    nc.scalar.activation(ta[:, H * h:H * (h + 1)], mm[:],
                                         AF.Tanh, bias=bi[:, 0:1])
                tas.append(ta)
                if i == 0 and inject is not None:
                    inject()
            # batched score matmuls: 4 col-tiles run concurrently on the PE
            for j in range(4):
                for i in range(GSZ):
                    r = 32 * i
                    nc.tensor.matmul(u1g[r:r + 1, bass.ts(j, 512)], va[:],
                                     tas[i][:, bass.ts(j, 512)], start=True,
                                     stop=True, tile_position=(0, r))
            st[g]["u1g"] = u1g

        def p1_exp(g):
            e1 = ep.tile([128, N], bf16, tag="e", name=f"e1_{g}")
            z1 = sp.tile([128, 1], f32, tag="z", name=f"z1_{g}")
            # nudge the Exp later in the scheduler's order so ready tanh work
            # fills its wait-for-last-score-matmul on the ACT FIFO
            with tc.high_priority(offset=-45):
                nc.scalar.activation(e1[:], st[g]["u1g"][:], AF.Exp,
                                     bias=bi[:, 2:3], accum_out=z1[:])
            st[g]["e1"], st[g]["z1"] = e1, z1

        def p1b(g):
            """Normalize a-weights and run the broadcast round-trip."""
            e1, z1 = st[g]["e1"], st[g]["z1"]
            r1 = sp.tile([128, 1], f32, tag="z", name=f"r1_{g}")
            nc.vector.reciprocal(r1[:], z1[:])
            an = ep.tile([128, N], bf16, tag="an", name=f"an{g}")
            nc.vector.tensor_scalar_mul(an[:], e1[:], r1[:])
            abc = []
            wr = []
            for i in range(GSZ):
                w = nc.sync.dma_start(scr_d.ap()[g * 8 + i:g * 8 + i + 1, :],
                                      an[32 * i:32 * i + 1, :])
                wr.append(w)
            for i in range(GSZ):
                t = bp.tile([128, N], bf16, tag="bc", name=f"abc{g}_{i}")
                rd = nc.sync.dma_start(
                    t[:],
                    scr_d.ap()[g * 8 + i:g * 8 + i + 1, :].partition_broadcast(128))
                bass_rust.add_dep_helper(rd.ins, wr[i].ins, sync=True,
                                         reason="scr RT RAW")
                abc.append(t)
            st[g]["abc"] = abc

        def p2_main(g, inject=None):
            """xa multiplies (then deferred y-work), branch-2 MMs/tanh/scores."""
            smp = list(range(g * GSZ, (g + 1) * GSZ))
            abc = st[g]["abc"]
            xas = []
            for i, s in enumerate(smp):
                xa = tp.tile([128, N], bf16, tag="xa", name=f"xa{s}")
                nc.vector.tensor_tensor(out=xa[:], in0=xts[s][:], in1=abc[i][:],
                                        op=ALU.mult)
                xas.append(xa)
            if g > 0:
                p3(g - 1)  # previous group's y-reduce: DVE work behind our xa
            u2g = pu.tile([128, N], f32, tag="u", name=f"u2g{g}")
            tgs = []
            for i, s in enumerate(smp):
                tg = tp.tile([128, N], bf16, tag="tanh", name=f"tg{s}")
                for h in range(2):
                    mm = pm.tile([128, H], f32, tag="mm", name=f"Gmm{s}_{h}")
                    for q in range(2):
                        sl = slice(H * h + 512 * q, H * h + 512 * (q + 1))
                        nc.tensor.matmul(mm[:, 512 * q:512 * (q + 1)], u1T[:],
                                         xts[s][:, sl], start=True, stop=False)
                        nc.tensor.matmul(mm[:, 512 * q:512 * (q + 1)], u2T[:],
                                         xas[i][:, sl], start=False, stop=True)
                    nc.scalar.activation(tg[:, H * h:H * (h + 1)], mm[:],
                                         AF.Tanh, bias=bi[:, 1:2])
                tgs.append(tg)
                if i == 0 and inject is not None:
                    inject()
            for j in range(4):
                for i in range(GSZ):
                    r = 32 * i
                    nc.tensor.matmul(u2g[r:r + 1, bass.ts(j, 512)], vc[:],
                                     tgs[i][:, bass.ts(j, 512)], start=True,
                                     stop=True, tile_position=(0, r))
            st[g]["u2g"] = u2g

        def p2_exp(g):
            e2 = ep.tile([128, N], bf16, tag="e", name=f"e2_{g}")
            z2 = sp.tile([128, 1], f32, tag="z", name=f"z2_{g}")
            off = -45 if g + 1 < NGROUPS else None
            if off is not None:
                with tc.high_priority(offset=off):
                    nc.scalar.activation(e2[:], st[g]["u2g"][:], AF.Exp,
                                         bias=bi[:, 3:4], accum_out=z2[:])
            else:
                nc.scalar.activation(e2[:], st[g]["u2g"][:], AF.Exp,
                                     bias=bi[:, 3:4], accum_out=z2[:])
            st[g]["e2"], st[g]["z2"] = e2, z2

        def p2b(g):
            """Normalize p-weights and run their broadcast round-trip."""
            e2, z2 = st[g]["e2"], st[g]["z2"]
            r2 = sp.tile([128, 1], f32, tag="z", name=f"r2_{g}")
            nc.vector.reciprocal(r2[:], z2[:])
            pn = ep.tile([128, N], bf16, tag="an", name=f"pn{g}")
            nc.vector.tensor_scalar_mul(pn[:], e2[:], r2[:])
            pbc = []
            wr = []
            for i in range(GSZ):
                w = nc.sync.dma_start(
                    scr_d.ap()[g * 8 + 4 + i:g * 8 + 4 + i + 1, :],
                    pn[32 * i:32 * i + 1, :])
                wr.append(w)
            for i in range(GSZ):
                t = bp.tile([128, N], bf16, tag="bc", name=f"pbc{g}_{i}")
                rd = nc.sync.dma_start(
                    t[:],
                    scr_d.ap()[g * 8 + 4 + i:g * 8 + 4 + i + 1, :]
                    .partition_broadcast(128))
                bass_rust.add_dep_helper(rd.ins, wr[i].ins, sync=True,
                                         reason="scr RT RAW")
                pbc.append(t)
            st[g]["pbc"] = pbc

        def p2b_last(g):
            """Last group: p-broadcast on the PE into freed PSUM, with 1/z
            folded into the broadcast matmul's stationary row."""
            e2, z2 = st[g]["e2"], st[g]["z2"]
            r2 = sp.tile([128, 1], f32, tag="z", name=f"r2_{g}")
            nc.vector.reciprocal(r2[:], z2[:])
            rzr = ep.tile([128, 128], bf16, tag="rzr", name=f"rzr{g}")
            nc.vector.tensor_scalar_mul(rzr[:], ones[:], r2[:])
            st[g]["rzr"] = rzr

        def p3_last(g):
            """Last group's y via PE row-broadcast + PSUM-side reduce."""
            smp = list(range(g * GSZ, (g + 1) * GSZ))
            e2, rzr = st[g]["e2"], st[g]["rzr"]
            ysa = sp.tile([128, GSZ], f32, tag="ysa", name="ysa")
            ysb = sp.tile([128, GSZ], f32, tag="ysb", name="ysb")
            for i, s in enumerate(smp):
                r = 32 * i
                for h in range(2):
                    pb = pm.tile([128, H], f32, tag="mm", name=f"pb{s}_{h}")
                    for q in range(2):
                        sl = slice(H * h + 512 * q, H * h + 512 * (q + 1))
                        nc.tensor.matmul(pb[:, 512 * q:512 * (q + 1)],
                                         rzr[r:r + 1, 0:128], e2[r:r + 1, sl],
                                         start=True, stop=True,
                                         tile_position=(r, 0))
                    jk = tp.tile([128, H], bf16, tag="jkh", bufs=2, name=f"jkh{s}_{h}")
                    acc = (ysa if h == 0 else ysb)
                    nc.vector.scalar_tensor_tensor(
                        jk[:], xts[s][:, H * h:H * (h + 1)], 1.0, pb[:],
                        ALU.mult, ALU.mult, accum_out=acc[:, i:i + 1])
            for i, s in enumerate(smp):
                nc.vector.tensor_add(ys[:, s:s + 1], ysa[:, i:i + 1], ysb[:, i:i + 1])

        def p3(g):
            """y = xT @ p (free-dim weighted reduce) for group g."""
            smp = list(range(g * GSZ, (g + 1) * GSZ))
            pbc = st[g]["pbc"]
            for i, s in enumerate(smp):
                jk = tp.tile([128, N], bf16, tag="xa", name=f"jk{s}")
                nc.vector.scalar_tensor_tensor(jk[:], xts[s][:], 1.0, pbc[i][:],
                                               ALU.mult, ALU.mult,
                                               accum_out=ys[:, s:s + 1])

        # software pipeline across groups
        load_x(0, split_first=True)
        u1T = cst.tile([S, E], bf16)
        nc.sync.dma_start(u1T[:], u1T_d.ap())
        u2T = cst.tile([S, E], bf16)
        nc.sync.dma_start(u2T[:], u2T_d.ap())
        va = cst.tile([E, 1], bf16)
        nc.sync.dma_start(va[:], va_d.ap())
        vc = cst.tile([E, 1], bf16)
        nc.sync.dma_start(vc[:], vc_d.ap())
        wsT = cst.tile([S, E], f32)
        nc.sync.dma_start(wsT[:], wsT_d.ap())
        w1T = cst.tile([E, E], f32)
        nc.sync.dma_start(w1T[:], w1T_d.ap())
        w2T = cst.tile([E, 1], f32)
        nc.sync.dma_start(w2T[:], w2T_d.ap())
        load_x(1)
        p1_main(0)
        p1_exp(0)
        p1b(0)
        for g in range(NGROUPS):
            if g + 2 < NGROUPS:
                load_x(g + 2)
            if g + 1 < NGROUPS:
                p1_main(g + 1)
                p1_exp(g + 1)
            p2_main(g)
            p2_exp(g)
            if g + 1 < NGROUPS:
                p1b(g + 1)
            if g == NGROUPS - 1:
                p2b_last(g)
            else:
                p2b(g)
        p3_last(NGROUPS - 1)

        # ---- head: v = W2 relu(W1 (W_s y + b_s) + b1) + b2 ----
        hp = pm.tile([128, BLOC], f32, tag="mm")
        nc.tensor.matmul(hp[:], wsT[:], ys[:], start=True, stop=True)
        hs = sp.tile([128, BLOC], f32, tag="hd")
        nc.vector.tensor_scalar_add(hs[:], hp[:], bi[:, 4:5])
        op_ = pm.tile([128, BLOC], f32, tag="mm")
        nc.tensor.matmul(op_[:], w1T[:], hs[:], start=True, stop=True)
        os_ = sp.tile([128, BLOC], f32, tag="hd")
        nc.vector.tensor_scalar(out=os_[:], in0=op_[:], scalar1=bi[:, 5:6],
                                scalar2=0.0, op0=ALU.add, op1=ALU.max)
        vp = pm.tile([128, BLOC], f32, tag="mm")
        nc.tensor.matmul(vp[0:1, :], w2T[:], os_[:], start=True, stop=True)
        vs = sp.tile([1, BLOC], f32, tag="vs")
        nc.vector.tensor_scalar_add(vs[:], vp[0:1, :], bi[0:1, 6:7])
        nc.sync.dma_start(v_out.ap(), vs[:])

    nc.compile()
    return nc


def kernel(instance, W_s, b_s, W_a, b_a, v_a, W_c, b_c, v_c, W1, b1, W2, b2):
    if "nc" not in _cache:
        _cache["nc"] = _build()
    nc = _cache["nc"]

    f64 = np.float64
    Ws, Wa, Wc = W_s.astype(f64), W_a.astype(f64), W_c.astype(f64)
    UA = Wa @ Ws
    U1 = Wc[:, :E].astype(f64) @ Ws
    U2 = Wc[:, E:].astype(f64) @ Ws
    biasA = Wa @ b_s.astype(f64) + b_a.astype(f64)
    biasG = Wc[:, :E] @ b_s.astype(f64) + b_c.astype(f64)
    bias2 = Wc[:, E:] @ b_s.astype(f64)
    assert np.abs(bias2).max() < 1e-12, "nonzero W_c2@b_s not supported"
    C1 = max(0.0, float(np.abs(v_a.astype(f64)).sum()) - 60.0)
    C2 = max(0.0, float(np.abs(v_c.astype(f64)).sum()) - 60.0)

    bi = np.zeros((128, 8), np.float32)
    bi[:, 0] = biasA
    bi[:, 1] = biasG
    bi[:, 2] = -C1
    bi[:, 3] = -C2
    bi[:, 4] = b_s
    bi[:, 5] = b1
    bi[0, 6] = float(b2[0])

    bcast = {
        "uaT": np.ascontiguousarray(UA.T).astype(ml_dtypes.bfloat16),
        "u1T": np.ascontiguousarray(U1.T).astype(ml_dtypes.bfloat16),
        "u2T": np.ascontiguousarray(U2.T).astype(ml_dtypes.bfloat16),
        "va": v_a.reshape(E, 1).astype(ml_dtypes.bfloat16),
        "vc": v_c.reshape(E, 1).astype(ml_dtypes.bfloat16),
        "wsT": np.ascontiguousarray(Ws.T).astype(np.float32),
        "w1T": np.ascontiguousarray(W1.astype(f64).T).astype(np.float32),
        "w2T": np.ascontiguousarray(W2.astype(f64).T).astype(np.float32),
        "bi": bi,
    }
    # host transpose: [B, N, S] -> per-core [BLOC, S, N] bf16, contiguous
    xb = np.asarray(instance).astype(ml_dtypes.bfloat16).transpose(0, 2, 1)
    in_maps = [dict(bcast, x=np.ascontiguousarray(xb[c * BLOC:(c + 1) * BLOC]))
               for c in range(NCORES)]
    _cache["in_maps"] = in_maps
    res = run_bass_kernel_spmd(nc, in_maps, core_ids=list(range(NCORES)))
    _cache["last_results"] = res
    return np.concatenate([res.results[c]["v"][0] for c in range(NCORES)]).astype(np.float32)


# revision 37
# speedup vs baseline: 1.1552x; 1.1552x over previous
"""Trainium2 Bass kernel for nn_Critic (additive-attention critic network).

Math (per sample, weight folding done on host):
  A    = UA @ x.T + biasA          UA = W_a@W_s           [E, N]
  u1   = v_a . tanh(A)                                    [N]
  a    = softmax(u1)  (constant-shift C1, exact softmax)
  G    = U1 @ x.T + U2 @ (x.T * a) + biasG                [E, N]
  u2   = v_c . tanh(G);  p = softmax(u2)
  y    = x.T @ p   (h_i = W_s@y + b_s since sum(p)=1)
  v    = W2 @ relu(W1 @ (W_s@y + b_s) + b1) + b2

Layout strategy:
  - x transposed on host to xT [S=128, N=2048] bf16 per sample: all
    device DMAs contiguous, no DMA transpose.
  - 4 samples per group; score rows at PSUM partitions {0,32,64,96} of
    one [128, N] f32 tile -> one batched Exp per group per branch.
  - Softmax-weight broadcasts via DRAM round-trip (row -> scratch DRAM ->
    stride-0 partition-broadcast read back): PSUM stays free for matmuls,
    DVE gets 2x bf16 SBUF mode for the xT*a multiply.
  - Groups are software-pipelined: branch-1 of group g+1 is emitted
    before branch-2 of group g so every engine FIFO (and every tile-pool
    ring) always has ready work while group g waits on its softmax
    round-trip.  Post-Exp DVE work (recip/normalize) is emitted after
    the next group's xa multiplies to keep the DVE FIFO unblocked.

Sharding: data-parallel over batch, 16 samples per core on 8 cores.
"""
import sys
import numpy as np

sys.path.insert(0, "/opt/trn_rl_repo")
import ml_dtypes  # noqa: E402
import concourse.bass as bass  # noqa: E402
import bass_rust  # noqa: E402
import concourse.bacc as bacc  # noqa: E402
import concourse.mybir as mybir  # noqa: E402
import concourse.tile as tile  # noqa: E402
from concourse.bass_utils import run_bass_kernel_spmd  # noqa: E402
from contextlib import ExitStack  # noqa: E402

B, N, S, E = 128, 2048, 128, 128
NCORES, BLOC = 8, 16
NGROUPS, GSZ = 4, 4
H = N // 2  # 1024
bf16, f32 = mybir.dt.bfloat16, mybir.dt.float32
AF, ALU = mybir.ActivationFunctionType, mybir.AluOpType

_cache = {}


def _build():
    nc = bacc.Bacc("TRN2", target_bir_lowering=False, debug=False, num_devices=NCORES)
    x_d = nc.dram_tensor("x", [BLOC, S, N], bf16, kind="ExternalInput")
    uaT_d = nc.dram_tensor("uaT", [S, E], bf16, kind="ExternalInput")
    u1T_d = nc.dram_tensor("u1T", [S, E], bf16, kind="ExternalInput")
    u2T_d = nc.dram_tensor("u2T", [S, E], bf16, kind="ExternalInput")
    va_d = nc.dram_tensor("va", [E, 1], bf16, kind="ExternalInput")
    vc_d = nc.dram_tensor("vc", [E, 1], bf16, kind="ExternalInput")
    wsT_d = nc.dram_tensor("wsT", [S, E], f32, kind="ExternalInput")
    w1T_d = nc.dram_tensor("w1T", [E, E], f32, kind="ExternalInput")
    w2T_d = nc.dram_tensor("w2T", [E, 1], f32, kind="ExternalInput")
    # bias columns: 0 biasA, 1 biasG, 2 -C1, 3 -C2, 4 b_s, 5 b1, 6 b2
    bi_d = nc.dram_tensor("bi", [128, 8], f32, kind="ExternalInput")
    # broadcast scratch: row (g*2+branch)*4 + i holds sample (4g+i)'s weights
    scr_d = nc.dram_tensor("scr", [NGROUPS * 2 * GSZ, N], bf16, kind="Internal")
    v_out = nc.dram_tensor("v", [1, BLOC], f32, kind="ExternalOutput")

    with tile.TileContext(nc) as tc, ExitStack() as ctx:
        cst = ctx.enter_context(tc.tile_pool(name="cst", bufs=1))
        xp = ctx.enter_context(tc.tile_pool(name="xp", bufs=1))
        tp = ctx.enter_context(tc.tile_pool(name="tp", bufs=6))
        bp = ctx.enter_context(tc.tile_pool(name="bp", bufs=9))
        ep = ctx.enter_context(tc.tile_pool(name="ep", bufs=2))
        sp = ctx.enter_context(tc.tile_pool(name="sp", bufs=8))
        pm = ctx.enter_context(tc.tile_pool(name="pm", bufs=2, space="PSUM"))
        pu = ctx.enter_context(tc.tile_pool(name="pu", bufs=1, space="PSUM"))

        # preload the exp/tanh ACT table set during the DMA ramp: first
        # activation in program order depends only on an on-chip memset
        warm0 = sp.tile([128, 1], f32, tag="z", name="warm0")
        warm1 = sp.tile([128, 1], f32, tag="z", name="warm1")
        nc.vector.memset(warm0[:], 0.0)
        nc.scalar.activation(warm1[:], warm0[:], AF.Tanh)
        ones = cst.tile([128, 128], bf16)
        nc.gpsimd.memset(ones[:], 1.0)

        uaT = cst.tile([S, E], bf16)
        nc.sync.dma_start(uaT[:], uaT_d.ap())
        bi = cst.tile([128, 8], f32)
        nc.sync.dma_start(bi[:], bi_d.ap())
        ys = cst.tile([128, BLOC], f32)

        xts = [None] * BLOC

        def load_x(g, split_first=False):
            for s in range(g * GSZ, (g + 1) * GSZ):
                xt = xp.tile([S, N], bf16, tag=f"x{s}", bufs=1, name=f"x{s}")
                if split_first and s == g * GSZ:
                    nc.sync.dma_start(xt[:, 0:H], x_d.ap()[s][:, 0:H])
                    nc.sync.dma_start(xt[:, H:N], x_d.ap()[s][:, H:N])
                else:
                    nc.sync.dma_start(xt[:], x_d.ap()[s])
                xts[s] = xt

        # per-group state carried between pipeline stages
        st = [dict() for _ in range(NGROUPS)]

        def p1_main(g, inject=None):
            """Branch 1 matmuls/tanh/scores for group g."""
            smp = list(range(g * GSZ, (g + 1) * GSZ))
            u1g = pu.tile([128, N], f32, tag="u", name=f"u1g{g}")
            tas = []
            for i, s in enumerate(smp):
                ta = tp.tile([128, N], bf16, tag="tanh", name=f"ta{s}")
                for h in range(2):
                    mm = pm.tile([128, H], f32, tag="mm", name=f"Amm{s}_{h}")
                    for q in range(2):
                        sl = slice(H * h + 512 * q, H * h + 512 * (q + 1))
                        nc.tensor.matmul(mm[:, 512 * q:512 * (q + 1)], uaT[:],
                                         xts[s][:, sl], start=True, stop=True)
                    nc.scalar.activation(ta[:, H * h:H * (h + 1)], mm[:],
                                         AF.Tanh, bias=bi[:, 0:1])
                tas.append(ta)
                if i == 0 and inject is not None:
                    inject()
            # batched score matmuls: 4 col-tiles run concurrently on the PE
            for j in range(4):
                for i in range(GSZ):
                    r = 32 * i
                    nc.tensor.matmul(u1g[r:r + 1, bass.ts(j, 512)], va[:],
                                     tas[i][:, bass.ts(j, 512)], start=True,
                                     stop=True, tile_position=(0, r))
            st[g]["u1g"] = u1g

        def p1_exp(g):
            e1 = ep.tile([128, N], bf16, tag="e", name=f"e1_{g}")
            z1 = sp.tile([128, 1], f32, tag="z", name=f"z1_{g}")
            # nudge the Exp later in the scheduler's order so ready tanh work
            # fills its wait-for-last-score-matmul on the ACT FIFO
            with tc.high_priority(offset=-45):
                nc.scalar.activation(e1[:], st[g]["u1g"][:], AF.Exp,
                                     bias=bi[:, 2:3], accum_out=z1[:])
            st[g]["e1"], st[g]["z1"] = e1, z1

        def p1b(g):
            """Normalize a-weights and run the broadcast round-trip."""
            e1, z1 = st[g]["e1"], st[g]["z1"]
            r1 = sp.tile([128, 1], f32, tag="z", name=f"r1_{g}")
            nc.vector.reciprocal(r1[:], z1[:])
            an = ep.tile([128, N], bf16, tag="an", name=f"an{g}")
            nc.vector.tensor_scalar_mul(an[:], e1[:], r1[:])
            abc = []
            wr = []
            for i in range(GSZ):
                w = nc.sync.dma_start(scr_d.ap()[g * 8 + i:g * 8 + i + 1, :],
                                      an[32 * i:32 * i + 1, :])
                wr.append(w)
            for i in range(GSZ):
                t = bp.tile([128, N], bf16, tag="bc", name=f"abc{g}_{i}")
                rd = nc.sync.dma_start(
                    t[:],
                    scr_d.ap()[g * 8 + i:g * 8 + i + 1, :].partition_broadcast(128))
                bass_rust.add_dep_helper(rd.ins, wr[i].ins, sync=True,
                                         reason="scr RT RAW")
                abc.append(t)
            st[g]["abc"] = abc

        def p2_main(g, inject=None):
            """xa multiplies (then deferred y-work), branch-2 MMs/tanh/scores."""
            smp = list(range(g * GSZ, (g + 1) * GSZ))
            abc = st[g]["abc"]
            xas = []
            for i, s in enumerate(smp):
                xa = tp.tile([128, N], bf16, tag="xa", name=f"xa{s}")
                nc.vector.tensor_tensor(out=xa[:], in0=xts[s][:], in1=abc[i][:],
                                        op=ALU.mult)
                xas.append(xa)
            if g > 0:
                p3(g - 1)  # previous group's y-reduce: DVE work behind our xa
            u2g = pu.tile([128, N], f32, tag="u", name=f"u2g{g}")
            tgs = []
            for i, s in enumerate(smp):
                tg = tp.tile([128, N], bf16, tag="tanh", name=f"tg{s}")
                for h in range(2):
                    mm = pm.tile([128, H], f32, tag="mm", name=f"Gmm{s}_{h}")
                    for q in range(2):
                        sl = slice(H * h + 512 * q, H * h + 512 * (q + 1))
                        nc.tensor.matmul(mm[:, 512 * q:512 * (q + 1)], u1T[:],
                                         xts[s][:, sl], start=True, stop=False)
                        nc.tensor.matmul(mm[:, 512 * q:512 * (q + 1)], u2T[:],
                                         xas[i][:, sl], start=False, stop=True)
                    nc.scalar.activation(tg[:, H * h:H * (h + 1)], mm[:],
                                         AF.Tanh, bias=bi[:, 1:2])
                tgs.append(tg)
                if i == 0 and inject is not None:
                    inject()
            for j in range(4):
                for i in range(GSZ):
                    r = 32 * i
                    nc.tensor.matmul(u2g[r:r + 1, bass.ts(j, 512)], vc[:],
                                     tgs[i][:, bass.ts(j, 512)], start=True,
                                     stop=True, tile_position=(0, r))
            st[g]["u2g"] = u2g

        def p2_exp(g):
            e2 = ep.tile([128, N], bf16, tag="e", name=f"e2_{g}")
            z2 = sp.tile([128, 1], f32, tag="z", name=f"z2_{g}")
            off = -45 if g + 1 < NGROUPS else None
            if off is not None:
                with tc.high_priority(offset=off):
                    nc.scalar.activation(e2[:], st[g]["u2g"][:], AF.Exp,
                                         bias=bi[:, 3:4], accum_out=z2[:])
            else:
                nc.scalar.activation(e2[:], st[g]["u2g"][:], AF.Exp,
                                     bias=bi[:, 3:4], accum_out=z2[:])
            st[g]["e2"], st[g]["z2"] = e2, z2

        def p2b(g):
            """Normalize p-weights and run their broadcast round-trip."""
            e2, z2 = st[g]["e2"], st[g]["z2"]
            r2 = sp.tile([128, 1], f32, tag="z", name=f"r2_{g}")
            nc.vector.reciprocal(r2[:], z2[:])
            pn = ep.tile([128, N], bf16, tag="an", name=f"pn{g}")
            nc.vector.tensor_scalar_mul(pn[:], e2[:], r2[:])
            pbc = []
            wr = []
            for i in range(GSZ):
                w = nc.sync.dma_start(
                    scr_d.ap()[g * 8 + 4 + i:g * 8 + 4 + i + 1, :],
                    pn[32 * i:32 * i + 1, :])
                wr.append(w)
            for i in range(GSZ):
                t = bp.tile([128, N], bf16, tag="bc", name=f"pbc{g}_{i}")
                rd = nc.sync.dma_start(
                    t[:],
                    scr_d.ap()[g * 8 + 4 + i:g * 8 + 4 + i + 1, :]
                    .partition_broadcast(128))
                bass_rust.add_dep_helper(rd.ins, wr[i].ins, sync=True,
                                         reason="scr RT RAW")
                pbc.append(t)
            st[g]["pbc"] = pbc

        def p2b_last(g):
            """Last group: normalize p, then RT-broadcast samples 1..3 while
            sample 0 will use a PE broadcast into freed PSUM."""
            e2, z2 = st[g]["e2"], st[g]["z2"]
            r2 = sp.tile([128, 1], f32, tag="z", name=f"r2_{g}")
            nc.vector.reciprocal(r2[:], z2[:])
            pn = ep.tile([128, N], bf16, tag="an", name=f"pn{g}")
            nc.vector.tensor_scalar_mul(pn[:], e2[:], r2[:])
            pbc = {}
            for i in range(1, GSZ):
                w = nc.sync.dma_start(
                    scr_d.ap()[g * 8 + 4 + i:g * 8 + 4 + i + 1, :],
                    pn[32 * i:32 * i + 1, :])
                t = bp.tile([128, N], bf16, tag="bc", name=f"pbc{g}_{i}")
                rd = nc.sync.dma_start(
                    t[:],
                    scr_d.ap()[g * 8 + 4 + i:g * 8 + 4 + i + 1, :]
                    .partition_broadcast(128))
                bass_rust.add_dep_helper(rd.ins, w.ins, sync=True,
                                         reason="scr RT RAW")
                pbc[i] = t
            st[g]["pn"], st[g]["pbc"] = pn, pbc

        def p3_last(g):
            """Last group's y split across engines: sample 0 via PE-broadcast
            + PSUM STT on the DVE; samples 1-3 via SBUF TT (2x DVE) with the
            reduction on the otherwise-idle ACT engine."""
            smp = list(range(g * GSZ, (g + 1) * GSZ))
            pn, pbc = st[g]["pn"], st[g]["pbc"]
            s0 = smp[0]
            ysa = sp.tile([128, 2], f32, tag="ysa", name="ysa")
            for h in range(2):
                pb = pm.tile([128, H], f32, tag="mm", name=f"pb{s0}_{h}")
                for q in range(2):
                    sl = slice(H * h + 512 * q, H * h + 512 * (q + 1))
                    nc.tensor.matmul(pb[:, 512 * q:512 * (q + 1)],
                                     ones[0:1, 0:128], pn[0:1, sl],
                                     start=True, stop=True, tile_position=(0, 0))
                jk = tp.tile([128, H], bf16, tag="jkh", bufs=2, name=f"jkh{s0}_{h}")
                nc.vector.scalar_tensor_tensor(
                    jk[:], xts[s0][:, H * h:H * (h + 1)], 1.0, pb[:],
                    ALU.mult, ALU.mult, accum_out=ysa[:, h:h + 1])
            nc.vector.tensor_add(ys[:, s0:s0 + 1], ysa[:, 0:1], ysa[:, 1:2])
            for i in range(1, GSZ):
                s = smp[i]
                jk = tp.tile([128, N], bf16, tag="xa", name=f"jk{s}")
                nc.vector.tensor_tensor(out=jk[:], in0=xts[s][:], in1=pbc[i][:],
                                        op=ALU.mult)
                jr = tp.tile([128, N], bf16, tag="jr", bufs=2, name=f"jr{s}")
                nc.scalar.activation(jr[:], jk[:], AF.Identity,
                                     accum_out=ys[:, s:s + 1])

        def p3(g):
            """y = xT @ p (free-dim weighted reduce) for group g."""
            smp = list(range(g * GSZ, (g + 1) * GSZ))
            pbc = st[g]["pbc"]
            for i, s in enumerate(smp):
                jk = tp.tile([128, N], bf16, tag="xa", name=f"jk{s}")
                nc.vector.scalar_tensor_tensor(jk[:], xts[s][:], 1.0, pbc[i][:],
                                               ALU.mult, ALU.mult,
                                               accum_out=ys[:, s:s + 1])

        # software pipeline across groups
        load_x(0, split_first=True)
        u1T = cst.tile([S, E], bf16)
        nc.sync.dma_start(u1T[:], u1T_d.ap())
        u2T = cst.tile([S, E], bf16)
        nc.sync.dma_start(u2T[:], u2T_d.ap())
        va = cst.tile([E, 1], bf16)
        nc.sync.dma_start(va[:], va_d.ap())
        vc = cst.tile([E, 1], bf16)
        nc.sync.dma_start(vc[:], vc_d.ap())
        wsT = cst.tile([S, E], f32)
        nc.sync.dma_start(wsT[:], wsT_d.ap())
        w1T = cst.tile([E, E], f32)
        nc.sync.dma_start(w1T[:], w1T_d.ap())
        w2T = cst.tile([E, 1], f32)
        nc.sync.dma_start(w2T[:], w2T_d.ap())
        load_x(1)
        p1_main(0)
        p1_exp(0)
        p1b(0)
        for g in range(NGROUPS):
            if g + 2 < NGROUPS:
                load_x(g + 2)
            if g + 1 < NGROUPS:
                p1_main(g + 1)
                p1_exp(g + 1)
            p2_main(g)
            p2_exp(g)
            if g + 1 < NGROUPS:
                p1b(g + 1)
            if g == NGROUPS - 1:
                p2b_last(g)
            else:
                p2b(g)
        p3_last(NGROUPS - 1)

        # ---- head: v = W2 relu(W1 (W_s y + b_s) + b1) + b2 ----
        hp = pm.tile([128, BLOC], f32, tag="mm")
        nc.tensor.matmul(hp[:], wsT[:], ys[:], start=True, stop=True)
        hs = sp.tile([128, BLOC], f32, tag="hd")
        nc.vector.tensor_scalar_add(hs[:], hp[:], bi[:, 4:5])
        op_ = pm.tile([128, BLOC], f32, tag="mm")
        nc.tensor.matmul(op_[:], w1T[:], hs[:], start=True, stop=True)
        os_ = sp.tile([128, BLOC], f32, tag="hd")
        nc.vector.tensor_scalar(out=os_[:], in0=op_[:], scalar1=bi[:, 5:6],
                                scalar2=0.0, op0=ALU.add, op1=ALU.max)
        vp = pm.tile([128, BLOC], f32, tag="mm")
        nc.tensor.matmul(vp[0:1, :], w2T[:], os_[:], start=True, stop=True)
        vs = sp.tile([1, BLOC], f32, tag="vs")
        nc.vector.tensor_scalar_add(vs[:], vp[0:1, :], bi[0:1, 6:7])
        nc.sync.dma_start(v_out.ap(), vs[:])

    nc.compile()
    return nc


def kernel(instance, W_s, b_s, W_a, b_a, v_a, W_c, b_c, v_c, W1, b1, W2, b2):
    if "nc" not in _cache:
        _cache["nc"] = _build()
    nc = _cache["nc"]

    f64 = np.float64
    Ws, Wa, Wc = W_s.astype(f64), W_a.astype(f64), W_c.astype(f64)
    UA = Wa @ Ws
    U1 = Wc[:, :E].astype(f64) @ Ws
    U2 = Wc[:, E:].astype(f64) @ Ws
    biasA = Wa @ b_s.astype(f64) + b_a.astype(f64)
    biasG = Wc[:, :E] @ b_s.astype(f64) + b_c.astype(f64)
    bias2 = Wc[:, E:] @ b_s.astype(f64)
    assert np.abs(bias2).max() < 1e-12, "nonzero W_c2@b_s not supported"
    C1 = max(0.0, float(np.abs(v_a.astype(f64)).sum()) - 60.0)
    C2 = max(0.0, float(np.abs(v_c.astype(f64)).sum()) - 60.0)

    bi = np.zeros((128, 8), np.float32)
    bi[:, 0] = biasA
    bi[:, 1] = biasG
    bi[:, 2] = -C1
    bi[:, 3] = -C2
    bi[:, 4] = b_s
    bi[:, 5] = b1
    bi[0, 6] = float(b2[0])

    bcast = {
        "uaT": np.ascontiguousarray(UA.T).astype(ml_dtypes.bfloat16),
        "u1T": np.ascontiguousarray(U1.T).astype(ml_dtypes.bfloat16),
        "u2T": np.ascontiguousarray(U2.T).astype(ml_dtypes.bfloat16),
        "va": v_a.reshape(E, 1).astype(ml_dtypes.bfloat16),
        "vc": v_c.reshape(E, 1).astype(ml_dtypes.bfloat16),
        "wsT": np.ascontiguousarray(Ws.T).astype(np.float32),
        "w1T": np.ascontiguousarray(W1.astype(f64).T).astype(np.float32),
        "w2T": np.ascontiguousarray(W2.astype(f64).T).astype(np.float32),
        "bi": bi,
    }
    # host transpose: [B, N, S] -> per-core [BLOC, S, N] bf16, contiguous
    xb = np.asarray(instance).astype(ml_dtypes.bfloat16).transpose(0, 2, 1)
    in_maps = [dict(bcast, x=np.ascontiguousarray(xb[c * BLOC:(c + 1) * BLOC]))
               for c in range(NCORES)]
    _cache["in_maps"] = in_maps
    res = run_bass_kernel_spmd(nc, in_maps, core_ids=list(range(NCORES)))
    _cache["last_results"] = res
    return np.concatenate([res.results[c]["v"][0] for c in range(NCORES)]).astype(np.float32)


# revision 38
# speedup vs baseline: 1.1776x; 1.0194x over previous
"""Trainium2 Bass kernel for nn_Critic (additive-attention critic network).

Math (per sample, weight folding done on host):
  A    = UA @ x.T + biasA          UA = W_a@W_s           [E, N]
  u1   = v_a . tanh(A)                                    [N]
  a    = softmax(u1)  (constant-shift C1, exact softmax)
  G    = U1 @ x.T + U2 @ (x.T * a) + biasG                [E, N]
  u2   = v_c . tanh(G);  p = softmax(u2)
  y    = x.T @ p   (h_i = W_s@y + b_s since sum(p)=1)
  v    = W2 @ relu(W1 @ (W_s@y + b_s) + b1) + b2

Layout strategy:
  - x transposed on host to xT [S=128, N=2048] bf16 per sample: all
    device DMAs contiguous, no DMA transpose.
  - 4 samples per group; score rows at PSUM partitions {0,32,64,96} of
    one [128, N] f32 tile -> one batched Exp per group per branch.
  - Softmax-weight broadcasts via DRAM round-trip (row -> scratch DRAM ->
    stride-0 partition-broadcast read back): PSUM stays free for matmuls,
    DVE gets 2x bf16 SBUF mode for the xT*a multiply.
  - Groups are software-pipelined: branch-1 of group g+1 is emitted
    before branch-2 of group g so every engine FIFO (and every tile-pool
    ring) always has ready work while group g waits on its softmax
    round-trip.  Post-Exp DVE work (recip/normalize) is emitted after
    the next group's xa multiplies to keep the DVE FIFO unblocked.

Sharding: data-parallel over batch, 16 samples per core on 8 cores.
"""
import sys
import numpy as np

sys.path.insert(0, "/opt/trn_rl_repo")
import ml_dtypes  # noqa: E402
import concourse.bass as bass  # noqa: E402
import bass_rust  # noqa: E402
import concourse.bacc as bacc  # noqa: E402
import concourse.mybir as mybir  # noqa: E402
import concourse.tile as tile  # noqa: E402
from concourse.bass_utils import run_bass_kernel_spmd  # noqa: E402
from contextlib import ExitStack  # noqa: E402

B, N, S, E = 128, 2048, 128, 128
NCORES, BLOC = 8, 16
NGROUPS, GSZ = 4, 4
H = N // 2  # 1024
bf16, f32 = mybir.dt.bfloat16, mybir.dt.float32
AF, ALU = mybir.ActivationFunctionType, mybir.AluOpType

_cache = {}


def _build():
    nc = bacc.Bacc("TRN2", target_bir_lowering=False, debug=False, num_devices=NCORES)
    x_d = nc.dram_tensor("x", [BLOC, S, N], bf16, kind="ExternalInput")
    uaT_d = nc.dram_tensor("uaT", [S, E], bf16, kind="ExternalInput")
    u1T_d = nc.dram_tensor("u1T", [S, E], bf16, kind="ExternalInput")
    u2T_d = nc.dram_tensor("u2T", [S, E], bf16, kind="ExternalInput")
    va_d = nc.dram_tensor("va", [E, 1], bf16, kind="ExternalInput")
    vc_d = nc.dram_tensor("vc", [E, 1], bf16, kind="ExternalInput")
    wsT_d = nc.dram_tensor("wsT", [S, E], f32, kind="ExternalInput")
    w1T_d = nc.dram_tensor("w1T", [E, E], f32, kind="ExternalInput")
    w2T_d = nc.dram_tensor("w2T", [E, 1], f32, kind="ExternalInput")
    # bias columns: 0 biasA, 1 biasG, 2 -C1, 3 -C2, 4 b_s, 5 b1, 6 b2
    bi_d = nc.dram_tensor("bi", [128, 8], f32, kind="ExternalInput")
    # broadcast scratch: row (g*2+branch)*4 + i holds sample (4g+i)'s weights
    scr_d = nc.dram_tensor("scr", [NGROUPS * 2 * GSZ, N], bf16, kind="Internal")
    v_out = nc.dram_tensor("v", [1, BLOC], f32, kind="ExternalOutput")

    with tile.TileContext(nc) as tc, ExitStack() as ctx:
        cst = ctx.enter_context(tc.tile_pool(name="cst", bufs=1))
        xp = ctx.enter_context(tc.tile_pool(name="xp", bufs=1))
        tp = ctx.enter_context(tc.tile_pool(name="tp", bufs=6))
        bp = ctx.enter_context(tc.tile_pool(name="bp", bufs=9))
        ep = ctx.enter_context(tc.tile_pool(name="ep", bufs=2))
        sp = ctx.enter_context(tc.tile_pool(name="sp", bufs=8))
        pm = ctx.enter_context(tc.tile_pool(name="pm", bufs=2, space="PSUM"))
        pu = ctx.enter_context(tc.tile_pool(name="pu", bufs=1, space="PSUM"))

        # preload the exp/tanh ACT table set during the DMA ramp: first
        # activation in program order depends only on an on-chip memset
        warm0 = sp.tile([128, 1], f32, tag="z", name="warm0")
        warm1 = sp.tile([128, 1], f32, tag="z", name="warm1")
        nc.vector.memset(warm0[:], 0.0)
        nc.scalar.activation(warm1[:], warm0[:], AF.Tanh)
        ones = cst.tile([128, 128], bf16)
        nc.gpsimd.memset(ones[:], 1.0)

        uaT = cst.tile([S, E], bf16)
        nc.sync.dma_start(uaT[:], uaT_d.ap())
        bi = cst.tile([128, 8], f32)
        nc.sync.dma_start(bi[:], bi_d.ap())
        ys = cst.tile([128, BLOC], f32)

        xts = [None] * BLOC

        def load_x(g, split_first=False):
            for s in range(g * GSZ, (g + 1) * GSZ):
                xt = xp.tile([S, N], bf16, tag=f"x{s}", bufs=1, name=f"x{s}")
                if split_first and s == g * GSZ:
                    nc.sync.dma_start(xt[:, 0:H], x_d.ap()[s][:, 0:H])
                    nc.sync.dma_start(xt[:, H:N], x_d.ap()[s][:, H:N])
                else:
                    nc.sync.dma_start(xt[:], x_d.ap()[s])
                xts[s] = xt

        # per-group state carried between pipeline stages
        st = [dict() for _ in range(NGROUPS)]

        def p1_main(g, inject=None):
            """Branch 1 matmuls/tanh/scores for group g."""
            smp = list(range(g * GSZ, (g + 1) * GSZ))
            u1g = pu.tile([128, N], f32, tag="u", name=f"u1g{g}")
            tas = []
            for i, s in enumerate(smp):
                ta = tp.tile([128, N], bf16, tag="tanh", name=f"ta{s}")
                for h in range(2):
                    mm = pm.tile([128, H], f32, tag="mm", name=f"Amm{s}_{h}")
                    for q in range(2):
                        sl = slice(H * h + 512 * q, H * h + 512 * (q + 1))
                        nc.tensor.matmul(mm[:, 512 * q:512 * (q + 1)], uaT[:],
                                         xts[s][:, sl], start=True, stop=True)
                    nc.scalar.activation(ta[:, H * h:H * (h + 1)], mm[:],
                                         AF.Tanh, bias=bi[:, 0:1])
                tas.append(ta)
                if i == 0 and inject is not None:
                    inject()
            # batched score matmuls: 4 col-tiles run concurrently on the PE
            for j in range(4):
                for i in range(GSZ):
                    r = 32 * i
                    nc.tensor.matmul(u1g[r:r + 1, bass.ts(j, 512)], va[:],
                                     tas[i][:, bass.ts(j, 512)], start=True,
                                     stop=True, tile_position=(0, r))
            st[g]["u1g"] = u1g

        def p1_exp(g):
            e1 = ep.tile([128, N], bf16, tag="e", name=f"e1_{g}")
            z1 = sp.tile([128, 1], f32, tag="z", name=f"z1_{g}")
            # nudge the Exp later in the scheduler's order so ready tanh work
            # fills its wait-for-last-score-matmul on the ACT FIFO
            with tc.high_priority(offset=-45):
                nc.scalar.activation(e1[:], st[g]["u1g"][:], AF.Exp,
                                     bias=bi[:, 2:3], accum_out=z1[:])
            st[g]["e1"], st[g]["z1"] = e1, z1

        def p1b(g):
            """Normalize a-weights and run the broadcast round-trip."""
            e1, z1 = st[g]["e1"], st[g]["z1"]
            r1 = sp.tile([128, 1], f32, tag="z", name=f"r1_{g}")
            nc.vector.reciprocal(r1[:], z1[:])
            an = ep.tile([128, N], bf16, tag="an", name=f"an{g}")
            nc.vector.tensor_scalar_mul(an[:], e1[:], r1[:])
            abc = []
            wr = []
            for i in range(GSZ):
                w = nc.sync.dma_start(scr_d.ap()[g * 8 + i:g * 8 + i + 1, :],
                                      an[32 * i:32 * i + 1, :])
                wr.append(w)
            for i in range(GSZ):
                t = bp.tile([128, N], bf16, tag="bc", name=f"abc{g}_{i}")
                rd = nc.sync.dma_start(
                    t[:],
                    scr_d.ap()[g * 8 + i:g * 8 + i + 1, :].partition_broadcast(128))
                bass_rust.add_dep_helper(rd.ins, wr[i].ins, sync=True,
                                         reason="scr RT RAW")
                abc.append(t)
            st[g]["abc"] = abc

        def p2_main(g, inject=None):
            """xa multiplies (then deferred y-work), branch-2 MMs/tanh/scores."""
            smp = list(range(g * GSZ, (g + 1) * GSZ))
            abc = st[g]["abc"]
            xas = []
            for i, s in enumerate(smp):
                xa = tp.tile([128, N], bf16, tag="xa", name=f"xa{s}")
                nc.vector.tensor_tensor(out=xa[:], in0=xts[s][:], in1=abc[i][:],
                                        op=ALU.mult)
                xas.append(xa)
            if g > 0:
                p3(g - 1)  # previous group's y-reduce: DVE work behind our xa
            u2g = pu.tile([128, N], f32, tag="u", name=f"u2g{g}")
            tgs = []
            for i, s in enumerate(smp):
                tg = tp.tile([128, N], bf16, tag="tanh", name=f"tg{s}")
                for h in range(2):
                    mm = pm.tile([128, H], f32, tag="mm", name=f"Gmm{s}_{h}")
                    for q in range(2):
                        sl = slice(H * h + 512 * q, H * h + 512 * (q + 1))
                        nc.tensor.matmul(mm[:, 512 * q:512 * (q + 1)], u1T[:],
                                         xts[s][:, sl], start=True, stop=False)
                        nc.tensor.matmul(mm[:, 512 * q:512 * (q + 1)], u2T[:],
                                         xas[i][:, sl], start=False, stop=True)
                    nc.scalar.activation(tg[:, H * h:H * (h + 1)], mm[:],
                                         AF.Tanh, bias=bi[:, 1:2])
                tgs.append(tg)
                if i == 0 and inject is not None:
                    inject()
            for j in range(4):
                for i in range(GSZ):
                    r = 32 * i
                    nc.tensor.matmul(u2g[r:r + 1, bass.ts(j, 512)], vc[:],
                                     tgs[i][:, bass.ts(j, 512)], start=True,
                                     stop=True, tile_position=(0, r))
            st[g]["u2g"] = u2g

        def p2_exp(g):
            e2 = ep.tile([128, N], bf16, tag="e", name=f"e2_{g}")
            z2 = sp.tile([128, 1], f32, tag="z", name=f"z2_{g}")
            off = -45 if g + 1 < NGROUPS else None
            if off is not None:
                with tc.high_priority(offset=off):
                    nc.scalar.activation(e2[:], st[g]["u2g"][:], AF.Exp,
                                         bias=bi[:, 3:4], accum_out=z2[:])
            else:
                nc.scalar.activation(e2[:], st[g]["u2g"][:], AF.Exp,
                                     bias=bi[:, 3:4], accum_out=z2[:])
            st[g]["e2"], st[g]["z2"] = e2, z2

        def p2b(g):
            """Normalize p-weights and run their broadcast round-trip."""
            e2, z2 = st[g]["e2"], st[g]["z2"]
            r2 = sp.tile([128, 1], f32, tag="z", name=f"r2_{g}")
            nc.vector.reciprocal(r2[:], z2[:])
            pn = ep.tile([128, N], bf16, tag="an", name=f"pn{g}")
            nc.vector.tensor_scalar_mul(pn[:], e2[:], r2[:])
            pbc = []
            wr = []
            for i in range(GSZ):
                w = nc.sync.dma_start(
                    scr_d.ap()[g * 8 + 4 + i:g * 8 + 4 + i + 1, :],
                    pn[32 * i:32 * i + 1, :])
                wr.append(w)
            for i in range(GSZ):
                t = bp.tile([128, N], bf16, tag="bc", name=f"pbc{g}_{i}")
                rd = nc.sync.dma_start(
                    t[:],
                    scr_d.ap()[g * 8 + 4 + i:g * 8 + 4 + i + 1, :]
                    .partition_broadcast(128))
                bass_rust.add_dep_helper(rd.ins, wr[i].ins, sync=True,
                                         reason="scr RT RAW")
                pbc.append(t)
            st[g]["pbc"] = pbc

        def p2b_last(g):
            """Last group: normalize p, then RT-broadcast samples 1..3 while
            sample 0 will use a PE broadcast into freed PSUM."""
            e2, z2 = st[g]["e2"], st[g]["z2"]
            r2 = sp.tile([128, 1], f32, tag="z", name=f"r2_{g}")
            nc.vector.reciprocal(r2[:], z2[:])
            pn = ep.tile([128, N], bf16, tag="an", name=f"pn{g}")
            nc.vector.tensor_scalar_mul(pn[:], e2[:], r2[:])
            pbc = {}
            for i in range(2, GSZ):
                w = nc.sync.dma_start(
                    scr_d.ap()[g * 8 + 4 + i:g * 8 + 4 + i + 1, :],
                    pn[32 * i:32 * i + 1, :])
                t = bp.tile([128, N], bf16, tag="bc", name=f"pbc{g}_{i}")
                rd = nc.sync.dma_start(
                    t[:],
                    scr_d.ap()[g * 8 + 4 + i:g * 8 + 4 + i + 1, :]
                    .partition_broadcast(128))
                bass_rust.add_dep_helper(rd.ins, w.ins, sync=True,
                                         reason="scr RT RAW")
                pbc[i] = t
            st[g]["pn"], st[g]["pbc"] = pn, pbc

        def p3_last(g):
            """Last group's y split across engines: sample 0 via PE-broadcast
            + PSUM STT on the DVE; samples 1-3 via SBUF TT (2x DVE) with the
            reduction on the otherwise-idle ACT engine."""
            smp = list(range(g * GSZ, (g + 1) * GSZ))
            pn, pbc = st[g]["pn"], st[g]["pbc"]
            ysa = sp.tile([128, 4], f32, tag="ysa", name="ysa")
            for i in range(2):
                s, r = smp[i], 32 * i
                for h in range(2):
                    pb = pm.tile([128, H], f32, tag="mm", name=f"pb{s}_{h}")
                    for q in range(2):
                        sl = slice(H * h + 512 * q, H * h + 512 * (q + 1))
                        nc.tensor.matmul(pb[:, 512 * q:512 * (q + 1)],
                                         ones[r:r + 1, 0:128], pn[r:r + 1, sl],
                                         start=True, stop=True,
                                         tile_position=(r, 0))
                    jk = tp.tile([128, H], bf16, tag="jkh", bufs=2,
                                 name=f"jkh{s}_{h}")
                    nc.vector.scalar_tensor_tensor(
                        jk[:], xts[s][:, H * h:H * (h + 1)], 1.0, pb[:],
                        ALU.mult, ALU.mult, accum_out=ysa[:, 2 * i + h:2 * i + h + 1])
            for i in range(2):
                s = smp[i]
                nc.vector.tensor_add(ys[:, s:s + 1], ysa[:, 2 * i:2 * i + 1],
                                     ysa[:, 2 * i + 1:2 * i + 2])
            for i in range(2, GSZ):
                s = smp[i]
                jk = tp.tile([128, N], bf16, tag="xa", name=f"jk{s}")
                nc.vector.tensor_tensor(out=jk[:], in0=xts[s][:], in1=pbc[i][:],
                                        op=ALU.mult)
                jr = tp.tile([128, N], bf16, tag="jr", bufs=2, name=f"jr{s}")
                nc.scalar.activation(jr[:], jk[:], AF.Identity,
                                     accum_out=ys[:, s:s + 1])

        def p3(g):
            """y = xT @ p (free-dim weighted reduce) for group g."""
            smp = list(range(g * GSZ, (g + 1) * GSZ))
            pbc = st[g]["pbc"]
            for i, s in enumerate(smp):
                jk = tp.tile([128, N], bf16, tag="xa", name=f"jk{s}")
                nc.vector.scalar_tensor_tensor(jk[:], xts[s][:], 1.0, pbc[i][:],
                                               ALU.mult, ALU.mult,
                                               accum_out=ys[:, s:s + 1])

        # software pipeline across groups
        load_x(0, split_first=True)
        u1T = cst.tile([S, E], bf16)
        nc.sync.dma_start(u1T[:], u1T_d.ap())
        u2T = cst.tile([S, E], bf16)
        nc.sync.dma_start(u2T[:], u2T_d.ap())
        va = cst.tile([E, 1], bf16)
        nc.sync.dma_start(va[:], va_d.ap())
        vc = cst.tile([E, 1], bf16)
        nc.sync.dma_start(vc[:], vc_d.ap())
        wsT = cst.tile([S, E], f32)
        nc.sync.dma_start(wsT[:], wsT_d.ap())
        w1T = cst.tile([E, E], f32)
        nc.sync.dma_start(w1T[:], w1T_d.ap())
        w2T = cst.tile([E, 1], f32)
        nc.sync.dma_start(w2T[:], w2T_d.ap())
        load_x(1)
        p1_main(0)
        p1_exp(0)
        p1b(0)
        for g in range(NGROUPS):
            if g + 2 < NGROUPS:
                load_x(g + 2)
            if g + 1 < NGROUPS:
                p1_main(g + 1)
                p1_exp(g + 1)
            p2_main(g)
            p2_exp(g)
            if g + 1 < NGROUPS:
                p1b(g + 1)
            if g == NGROUPS - 1:
                p2b_last(g)
            else:
                p2b(g)
        p3_last(NGROUPS - 1)

        # ---- head: v = W2 relu(W1 (W_s y + b_s) + b1) + b2 ----
        hp = pm.tile([128, BLOC], f32, tag="mm")
        nc.tensor.matmul(hp[:], wsT[:], ys[:], start=True, stop=True)
        hs = sp.tile([128, BLOC], f32, tag="hd")
        nc.vector.tensor_scalar_add(hs[:], hp[:], bi[:, 4:5])
        op_ = pm.tile([128, BLOC], f32, tag="mm")
        nc.tensor.matmul(op_[:], w1T[:], hs[:], start=True, stop=True)
        os_ = sp.tile([128, BLOC], f32, tag="hd")
        nc.vector.tensor_scalar(out=os_[:], in0=op_[:], scalar1=bi[:, 5:6],
                                scalar2=0.0, op0=ALU.add, op1=ALU.max)
        vp = pm.tile([128, BLOC], f32, tag="mm")
        nc.tensor.matmul(vp[0:1, :], w2T[:], os_[:], start=True, stop=True)
        vs = sp.tile([1, BLOC], f32, tag="vs")
        nc.vector.tensor_scalar_add(vs[:], vp[0:1, :], bi[0:1, 6:7])
        nc.sync.dma_start(v_out.ap(), vs[:])

    nc.compile()
    return nc


def kernel(instance, W_s, b_s, W_a, b_a, v_a, W_c, b_c, v_c, W1, b1, W2, b2):
    if "nc" not in _cache:
        _cache["nc"] = _build()
    nc = _cache["nc"]

    f64 = np.float64
    Ws, Wa, Wc = W_s.astype(f64), W_a.astype(f64), W_c.astype(f64)
    UA = Wa @ Ws
    U1 = Wc[:, :E].astype(f64) @ Ws
    U2 = Wc[:, E:].astype(f64) @ Ws
    biasA = Wa @ b_s.astype(f64) + b_a.astype(f64)
    biasG = Wc[:, :E] @ b_s.astype(f64) + b_c.astype(f64)
    bias2 = Wc[:, E:] @ b_s.astype(f64)
    assert np.abs(bias2).max() < 1e-12, "nonzero W_c2@b_s not supported"
    C1 = max(0.0, float(np.abs(v_a.astype(f64)).sum()) - 60.0)
    C2 = max(0.0, float(np.abs(v_c.astype(f64)).sum()) - 60.0)

    bi = np.zeros((128, 8), np.float32)
    bi[:, 0] = biasA
    bi[:, 1] = biasG
    bi[:, 2] = -C1
    bi[:, 3] = -C2
    bi[:, 4] = b_s
    bi[:, 5] = b1
    bi[0, 6] = float(b2[0])

    bcast = {
        "uaT": np.ascontiguousarray(UA.T).astype(ml_dtypes.bfloat16),
        "u1T": np.ascontiguousarray(U1.T).astype(ml_dtypes.bfloat16),
        "u2T": np.ascontiguousarray(U2.T).astype(ml_dtypes.bfloat16),
        "va": v_a.reshape(E, 1).astype(ml_dtypes.bfloat16),
        "vc": v_c.reshape(E, 1).astype(ml_dtypes.bfloat16),
        "wsT": np.ascontiguousarray(Ws.T).astype(np.float32),
        "w1T": np.ascontiguousarray(W1.astype(f64).T).astype(np.float32),
        "w2T": np.ascontiguousarray(W2.astype(f64).T).astype(np.float32),
        "bi": bi,
    }
    # host transpose: [B, N, S] -> per-core [BLOC, S, N] bf16, contiguous
    xb = np.asarray(instance).astype(ml_dtypes.bfloat16).transpose(0, 2, 1)
    in_maps = [dict(bcast, x=np.ascontiguousarray(xb[c * BLOC:(c + 1) * BLOC]))
               for c in range(NCORES)]
    _cache["in_maps"] = in_maps
    res = run_bass_kernel_spmd(nc, in_maps, core_ids=list(range(NCORES)))
    _cache["last_results"] = res
    return np.concatenate([res.results[c]["v"][0] for c in range(NCORES)]).astype(np.float32)
